# revision 3
# baseline (speedup 1.0000x reference)
"""Multi-head attention kernel for Trainium2, sharded over 8 NeuronCores.

Full inputs q,k,v: [2, 16, 2048, 64] fp32. Heads (B*H = 32) are sharded 4 per
core; each core runs flash-attention-style softmax(Q K^T / sqrt(d)) V for its
heads with no cross-core communication.

Per-head scheme (n=2048, d=64), all on-chip:
  - Load Q,K,V; PE-transpose Q,K into Q^T,K^T [64, 2048] (f32r).
  - For each 1024-wide query block, loop over 16 key chunks j:
      S^T_j = K_j @ Q^T        (PE, f32r, [128, 1024] PSUM)
      P^T_j = exp(0.125*S^T_j) (ACT, PSUM->SBUF, f32r)
      out^T += [V_j | 1]^T @ P^T_j  (PE accumulate, [65, 1024] PSUM;
                                     row 64 = softmax denominator)
  - Finalize: PE-transpose out^T back to [i, d] chunks, multiply by the
    per-row reciprocal denominator (DVE), DMA out.
No max-subtraction is needed: scores are N(0,1)-scaled, |S|<~9, exp is safe
in fp32.
"""

import sys

sys.path.insert(0, "/opt/trn_rl_repo")

import numpy as np

import concourse.bass as bass
import concourse.mybir as mybir
import concourse.tile as tile
from concourse import bacc
from concourse.bass_utils import run_bass_kernel_spmd
from concourse.masks import make_identity

B, H, N, D = 2, 16, 2048, 64
NCORES = 8
HPC = (B * H) // NCORES  # heads per core
SCALE = float(D) ** -0.5

F32 = mybir.dt.float32
F32R = mybir.dt.float32r
EXP = mybir.ActivationFunctionType.Exp

NJ = N // 128  # 16 key chunks of 128
IB = 1024  # query-block width (PSUM: [128, IB] = 2 banks)
NIB = N // IB


def _emit(tc):
    nc = tc.nc
    q_d = nc.dram_tensor("q", [HPC, N, D], F32, kind="ExternalInput").ap()
    k_d = nc.dram_tensor("k", [HPC, N, D], F32, kind="ExternalInput").ap()
    v_d = nc.dram_tensor("v", [HPC, N, D], F32, kind="ExternalInput").ap()
    o_d = nc.dram_tensor("o", [HPC, N, D], F32, kind="ExternalOutput").ap()

    from contextlib import ExitStack

    with ExitStack() as ctx:
        stg = ctx.enter_context(tc.tile_pool(name="stg", bufs=2))
        persist = ctx.enter_context(tc.tile_pool(name="persist", bufs=2))
        pt_pool = ctx.enter_context(tc.tile_pool(name="pt", bufs=3))
        osb_pool = ctx.enter_context(tc.tile_pool(name="osb", bufs=2))
        fin_pool = ctx.enter_context(tc.tile_pool(name="fin", bufs=3))
        const_pool = ctx.enter_context(tc.tile_pool(name="const", bufs=1))
        st_pool = ctx.enter_context(tc.tile_pool(name="st", bufs=2, space="PSUM"))
        ot_pool = ctx.enter_context(tc.tile_pool(name="ot", bufs=1, space="PSUM"))
        tr_pool = ctx.enter_context(tc.tile_pool(name="tr", bufs=2, space="PSUM"))

        ident = const_pool.tile([128, 128], F32)
        make_identity(nc, ident[:])
        ones_stg = const_pool.tile([128, NJ], F32)
        nc.gpsimd.memset(ones_stg[:], 1.0)

        for h in range(HPC):
            # ---- Phase 1: load + transpose inputs ----
            q_stg = stg.tile([128, NJ, D], F32, tag="qstg")
            nc.sync.dma_start(q_stg[:], q_d[h].rearrange("(t p) d -> p t d", p=128))
            k_stg = stg.tile([128, NJ, D], F32, tag="kstg")
            nc.sync.dma_start(k_stg[:], k_d[h].rearrange("(t p) d -> p t d", p=128))
            v_stg = stg.tile([128, NJ, D], F32, tag="vstg")
            nc.sync.dma_start(v_stg[:], v_d[h].rearrange("(t p) d -> p t d", p=128))

            qt = persist.tile([D, N], F32R, tag="qt")
            kt = persist.tile([D, N], F32R, tag="kt")
            vones = persist.tile([128, NJ, D + 1], F32R, tag="vones")

            for src, dst in ((q_stg, qt), (k_stg, kt)):
                for g in range(NJ // 4):  # 4 transposes per PSUM bank
                    tr = tr_pool.tile([D, 512], F32, tag="tr")
                    for u in range(4):
                        t = 4 * g + u
                        nc.tensor.transpose(
                            tr[:, u * 128 : (u + 1) * 128],
                            src[:, t, :],
                            ident[:],
                        )
                    nc.vector.tensor_copy(dst[:, g * 512 : (g + 1) * 512], tr[:])
            for t in range(NJ):
                nc.vector.tensor_copy(vones[:, t, 0:D], v_stg[:, t, :])
            nc.vector.tensor_copy(vones[:, :, D], ones_stg[:])

            # ---- Phase 2: attention ----
            for ib in range(NIB):
                ot = ot_pool.tile([D + 1, IB], F32, tag="ot")
                for j in range(NJ):
                    st = st_pool.tile([128, IB], F32, tag="st")
                    for hh in range(IB // 512):
                        nc.tensor.matmul(
                            st[:, hh * 512 : (hh + 1) * 512],
                            kt[:, j * 128 : (j + 1) * 128],
                            qt[:, ib * IB + hh * 512 : ib * IB + (hh + 1) * 512],
                            start=True,
                            stop=True,
                        )
                    pt = pt_pool.tile([128, IB], F32R, tag="pt")
                    nc.scalar.activation(pt[:], st[:], EXP, scale=SCALE)
                    for hh in range(IB // 512):
                        nc.tensor.matmul(
                            ot[:, hh * 512 : (hh + 1) * 512],
                            vones[:, j, :],
                            pt[:, hh * 512 : (hh + 1) * 512],
                            start=(j == 0),
                            stop=(j == NJ - 1),
                        )

                # ---- Finalize this query block ----
                osb = osb_pool.tile([D + 1, IB], F32, tag="osb")
                nc.vector.tensor_copy(osb[:], ot[:])
                for t in range(IB // 128):
                    trf = tr_pool.tile([128, D + 1], F32, tag="tr")
                    nc.tensor.transpose(
                        trf[:],
                        osb[:, t * 128 : (t + 1) * 128],
                        ident[0 : D + 1, 0 : D + 1],
                    )
                    fin = fin_pool.tile([128, D + 1], F32, tag="fin")
                    nc.vector.reciprocal(fin[:, D : D + 1], trf[:, D : D + 1])
                    nc.vector.tensor_scalar_mul(
                        fin[:, 0:D], trf[:, 0:D], fin[:, D : D + 1]
                    )
                    nc.sync.dma_start(
                        o_d[h, ib * IB + t * 128 : ib * IB + (t + 1) * 128, :],
                        fin[:, 0:D],
                    )


_CACHE = {}


def _build():
    if "nc" in _CACHE:
        return _CACHE["nc"]
    nc = bacc.Bacc("TRN2", target_bir_lowering=False, debug=False, num_devices=NCORES)
    with tile.TileContext(nc) as tc:
        _emit(tc)
    nc.compile()
    _CACHE["nc"] = nc
    return nc


def run(q, k, v, trace=False, **spmd_kwargs):
    nc = _build()
    qf = np.ascontiguousarray(np.asarray(q, dtype=np.float32).reshape(B * H, N, D))
    kf = np.ascontiguousarray(np.asarray(k, dtype=np.float32).reshape(B * H, N, D))
    vf = np.ascontiguousarray(np.asarray(v, dtype=np.float32).reshape(B * H, N, D))
    in_maps = [
        {
            "q": qf[c * HPC : (c + 1) * HPC],
            "k": kf[c * HPC : (c + 1) * HPC],
            "v": vf[c * HPC : (c + 1) * HPC],
        }
        for c in range(NCORES)
    ]
    res = run_bass_kernel_spmd(nc, in_maps, list(range(NCORES)), trace=trace, **spmd_kwargs)
    out = np.concatenate([res.results[c]["o"] for c in range(NCORES)], axis=0)
    return out.reshape(B, H, N, D).astype(np.float32), res


def kernel(q, k, v):
    out, _ = run(q, k, v)
    return out
